# revision 10
# baseline (speedup 1.0000x reference)
"""HGAC loss kernel for 8 Trainium2 NeuronCores.

Layout of the computation:
  - Data-parallel over the batch dim B=128: each of the 8 cores handles 16
    batches of the node similarity matrices (V@T^T), per-node squared norms,
    and the graph-MSE partial sums.  The global InfoNCE stats are computed
    (replicated) on every core from the full v_g/t_g.
  - Hungarian solves are inherently sequential/branchy -> solved on host from
    the device-produced cost matrices (the reference does the same: its
    Hungarian runs in numpy on host).
  - L_node uses the permutation identity:
        mean((V - T[cols])^2) = (sum V^2 + sum T^2 - 2*sum_{b,n} S[b,n,cols])
                                 / (B*N*D)
    since cols is a permutation per batch, so only S = V@T^T (unnormalized),
    row squared-norms and the assignment are needed.
"""

import os
import sys
import numpy as np
from concurrent.futures import ProcessPoolExecutor

sys.path.insert(0, "/opt/trn_rl_repo")
sys.path.insert(0, "/opt/pypackages")

B, N, D = 128, 128, 1024
NCORES = 8
BLOC = B // NCORES  # 16 batches per core
TEMP = 0.07
KCH = D // 128  # 8 contraction chunks

_CACHE = {}


def _build_program():
    import concourse.bass as bass
    import concourse.tile as tile
    from concourse import bacc, masks, mybir

    f32 = mybir.dt.float32
    bf16 = mybir.dt.bfloat16
    AF = mybir.ActivationFunctionType
    ALU = mybir.AluOpType

    nc = bacc.Bacc(
        "TRN2",
        target_bir_lowering=False,
        debug=False,
    )

    v_loc = nc.declare_dram_parameter("v_loc", [BLOC, N, D], f32, isOutput=False)
    t_loc = nc.declare_dram_parameter("t_loc", [BLOC, N, D], f32, isOutput=False)
    av_loc = nc.declare_dram_parameter("av_loc", [BLOC, N, N], f32, isOutput=False)
    at_loc = nc.declare_dram_parameter("at_loc", [BLOC, N, N], f32, isOutput=False)
    vg = nc.declare_dram_parameter("vg", [B, D], f32, isOutput=False)
    tg = nc.declare_dram_parameter("tg", [B, D], f32, isOutput=False)
    rv = nc.declare_dram_parameter("rv", [B, 1], f32, isOutput=False)
    rt = nc.declare_dram_parameter("rt", [B, 1], f32, isOutput=False)

    s_out = nc.declare_dram_parameter("s_out", [N, BLOC * N], f32, isOutput=True)
    rsq_out = nc.declare_dram_parameter("rsq_out", [N, 2 * BLOC], f32, isOutput=True)
    apart_out = nc.declare_dram_parameter("apart_out", [N, 2], f32, isOutput=True)
    nce_out = nc.declare_dram_parameter("nce_out", [B, 4], f32, isOutput=True)
    sim_out = nc.declare_dram_parameter("sim_out", [B, B], f32, isOutput=True)

    with tile.TileContext(nc) as tc:
        with (
            tc.tile_pool(name="const", bufs=1) as constp,
            tc.tile_pool(name="vin", bufs=4) as vinp,
            tc.tile_pool(name="vt", bufs=4) as vtp,
            tc.tile_pool(name="sqscr", bufs=2) as sqscrp,
            tc.tile_pool(name="sgrp", bufs=3) as sgrpp,
            tc.tile_pool(name="stats", bufs=1) as statsp,
            tc.tile_pool(name="apool", bufs=2) as apool,
            tc.tile_pool(name="nce", bufs=1) as ncep,
            tc.tile_pool(name="trps", bufs=4, space="PSUM") as trpsp,
            tc.tile_pool(name="spsum", bufs=3, space="PSUM") as spsump,
        ):
            ident_bf = constp.tile([128, 128], bf16)
            masks.make_identity(nc, ident_bf[:])
            ident_f32 = constp.tile([128, 128], f32)
            masks.make_identity(nc, ident_f32[:])

            # ---------------- global InfoNCE (replicated on all cores) -------
            vg_t = ncep.tile([B, D], f32, tag="vg")
            tg_t = ncep.tile([B, D], f32, tag="tg")
            rv_t = ncep.tile([B, 1], f32, tag="rv")
            rt_t = ncep.tile([B, 1], f32, tag="rt")
            nc.sync.dma_start(vg_t[:], vg[:])
            nc.sync.dma_start(tg_t[:], tg[:])
            nc.sync.dma_start(rv_t[:], rv[:])
            nc.sync.dma_start(rt_t[:], rt[:])

            vn = ncep.tile([B, D], f32, tag="vn")
            tn = ncep.tile([B, D], f32, tag="tn")
            nc.scalar.activation(vn[:], vg_t[:], AF.Copy, scale=rv_t[:])
            nc.scalar.activation(tn[:], tg_t[:], AF.Copy, scale=rt_t[:])

            vnT = ncep.tile([B, D], f32, tag="vnT")
            tnT = ncep.tile([B, D], f32, tag="tnT")
            for src, dst in ((vn, vnT), (tn, tnT)):
                for k in range(KCH):
                    ps = spsump.tile([128, 128], f32, tag="sps")
                    nc.tensor.transpose(
                        ps[:], src[:, k * 128 : (k + 1) * 128], ident_f32[:]
                    )
                    nc.vector.tensor_copy(dst[:, k * 128 : (k + 1) * 128], ps[:])

            sim_ps = spsump.tile([128, 128], f32, tag="sps")
            for k in range(KCH):
                nc.tensor.matmul(
                    sim_ps[:],
                    vnT[:, k * 128 : (k + 1) * 128],
                    tnT[:, k * 128 : (k + 1) * 128],
                    start=(k == 0),
                    stop=(k == KCH - 1),
                )
            sim = ncep.tile([B, B], f32, tag="sim")
            nc.scalar.activation(sim[:], sim_ps[:], AF.Copy, scale=1.0 / TEMP)
            nc.sync.dma_start(sim_out[:], sim[:])

            nce_t = statsp.tile([B, 4], f32, tag="nce")
            nc.gpsimd.memset(nce_t[:], 0.0)
            scrE = ncep.tile([B, B], f32, tag="scrE")
            # rows of sim (i2t) and rows of sim.T (t2i)
            for direc, src in ((0, sim), (2, None)):
                if src is None:
                    ps = spsump.tile([128, 128], f32, tag="sps")
                    nc.tensor.transpose(ps[:], sim[:], ident_f32[:])
                    src = ncep.tile([B, B], f32, tag="simT")
                    nc.vector.tensor_copy(src[:], ps[:])
                m = ncep.tile([B, 1], f32, tag=f"m{direc}")
                nc.vector.reduce_max(m[:], src[:], axis=mybir.AxisListType.X)
                negm = ncep.tile([B, 1], f32, tag=f"negm{direc}")
                nc.scalar.mul(negm[:], m[:], -1.0)
                sumE = ncep.tile([B, 1], f32, tag=f"sumE{direc}")
                nc.scalar.activation(
                    scrE[:], src[:], AF.Exp, bias=negm[:], accum_out=sumE[:]
                )
                lgs = ncep.tile([B, 1], f32, tag=f"lgs{direc}")
                nc.scalar.activation(lgs[:], sumE[:], AF.Ln)
                # lse = m + log(sum(exp(x - m)))
                nc.vector.tensor_add(nce_t[:, direc : direc + 1], m[:], lgs[:])
            # diagonal of sim via identity mask multiply then row-reduce
            scrD = ncep.tile([B, B], f32, tag="scrD")
            nc.vector.tensor_mul(scrD[:], sim[:], ident_f32[:])
            nc.vector.reduce_sum(
                nce_t[:, 1:2], scrD[:], axis=mybir.AxisListType.X
            )
            nc.sync.dma_start(nce_out[:], nce_t[:])

            # ---------------- node term: per-batch S = V @ T^T ---------------
            rsq_t_all = statsp.tile([N, 2 * BLOC], f32, tag="rsq")
            for b in range(BLOC):
                vbf = vinp.tile([N, D], bf16, tag="vbf")
                tbf = vinp.tile([N, D], bf16, tag="tbf")
                nc.gpsimd.dma_start(vbf[:], v_loc[b])  # f32 -> bf16 cast DMA
                nc.gpsimd.dma_start(tbf[:], t_loc[b])

                sqv = sqscrp.tile([N, D], bf16, tag="sq")
                nc.scalar.activation(
                    sqv[:], vbf[:], AF.Square, accum_out=rsq_t_all[:, b : b + 1]
                )
                sqt = sqscrp.tile([N, D], bf16, tag="sq")
                nc.scalar.activation(
                    sqt[:],
                    tbf[:],
                    AF.Square,
                    accum_out=rsq_t_all[:, BLOC + b : BLOC + b + 1],
                )

                trv = trpsp.tile([128, D], bf16, tag="tr")
                trt = trpsp.tile([128, D], bf16, tag="tr")
                for k in range(KCH):
                    nc.tensor.transpose(
                        trv[:, k * 128 : (k + 1) * 128],
                        vbf[:, k * 128 : (k + 1) * 128],
                        ident_bf[:],
                    )
                    nc.tensor.transpose(
                        trt[:, k * 128 : (k + 1) * 128],
                        tbf[:, k * 128 : (k + 1) * 128],
                        ident_bf[:],
                    )
                vt = vtp.tile([128, D], bf16, tag="vt")
                tt = vtp.tile([128, D], bf16, tag="tt")
                nc.vector.tensor_copy(vt[:], trv[:])
                nc.scalar.copy(tt[:], trt[:])

                sps = spsump.tile([128, 128], f32, tag="sps")
                for k in range(KCH):
                    nc.tensor.matmul(
                        sps[:],
                        vt[:, k * 128 : (k + 1) * 128],
                        tt[:, k * 128 : (k + 1) * 128],
                        start=(k == 0),
                        stop=(k == KCH - 1),
                    )
                g, i = divmod(b, 4)
                if i == 0:
                    sgrp = sgrpp.tile([128, 512], f32, tag="sgrp")
                nc.vector.tensor_copy(sgrp[:, i * 128 : (i + 1) * 128], sps[:])
                if i == 3:
                    nc.sync.dma_start(s_out[:, g * 512 : (g + 1) * 512], sgrp[:])
            nc.sync.dma_start(rsq_out[:], rsq_t_all[:])

            # ---------------- graph term partials ----------------------------
            apart_t = statsp.tile([N, 2], f32, tag="apart")
            av_r = av_loc[:].rearrange("b p f -> p b f")
            at_r = at_loc[:].rearrange("b p f -> p b f")
            for j in range(2):
                av_t = apool.tile([128, 8 * N], f32, tag="av")
                at_t = apool.tile([128, 8 * N], f32, tag="at")
                nc.sync.dma_start(av_t[:], av_r[:, j * 8 : (j + 1) * 8, :])
                nc.sync.dma_start(at_t[:], at_r[:, j * 8 : (j + 1) * 8, :])
                dif = apool.tile([128, 8 * N], f32, tag="dif")
                nc.vector.tensor_sub(dif[:], av_t[:], at_t[:])
                dsq = sqscrp.tile([128, 8 * N], bf16, tag="sq")
                nc.scalar.activation(
                    dsq[:], dif[:], AF.Square, accum_out=apart_t[:, j : j + 1]
                )
            nc.sync.dma_start(apart_out[:], apart_t[:])

    nc.compile()
    return nc


def _get_program():
    if "nc" not in _CACHE:
        _CACHE["nc"] = _build_program()
    return _CACHE["nc"]


def _hungarian(cost):
    """Min-cost assignment; identical to the reference implementation."""
    n = cost.shape[0]
    u = np.zeros(n + 1)
    v = np.zeros(n + 1)
    p = np.zeros(n + 1, dtype=np.int64)
    way = np.zeros(n + 1, dtype=np.int64)
    for i in range(1, n + 1):
        p[0] = i
        j0 = 0
        minv = np.full(n + 1, np.inf)
        used = np.zeros(n + 1, dtype=bool)
        while True:
            used[j0] = True
            i0 = p[j0]
            cur = cost[i0 - 1] - u[i0] - v[1:]
            mask = ~used[1:]
            upd = mask & (cur < minv[1:])
            minv[1:][upd] = cur[upd]
            way[1:][upd] = j0
            cand = np.where(mask, minv[1:], np.inf)
            j1 = int(np.argmin(cand)) + 1
            delta = cand[j1 - 1]
            u[p[used]] += delta
            v[used] -= delta
            minv[~used] -= delta
            j0 = j1
            if p[j0] == 0:
                break
        while j0 != 0:
            j1 = way[j0]
            p[j0] = p[j1]
            j0 = j1
    col_for_row = np.zeros(n, dtype=np.int64)
    for j in range(1, n + 1):
        col_for_row[p[j] - 1] = j - 1
    return col_for_row


def _hung_batch(costs):
    return np.stack([_hungarian(c) for c in costs])


def _solve_hungarian(cost):
    """cost: [B, N, N] float64 -> cols [B, N] int64, parallel over processes."""
    nw = min(16, os.cpu_count() or 1)
    if nw <= 1:
        return _hung_batch(cost)
    chunks = np.array_split(cost, nw)
    try:
        import multiprocessing as mp

        ctx = mp.get_context("fork")
        with ProcessPoolExecutor(max_workers=nw, mp_context=ctx) as ex:
            parts = list(ex.map(_hung_batch, chunks))
        return np.concatenate(parts)
    except Exception:
        return _hung_batch(cost)


def kernel(v_g, t_g, V, T, A_v, A_t):
    from concourse.bass_utils import run_bass_kernel_spmd

    nc = _get_program()

    v_g = np.ascontiguousarray(v_g, dtype=np.float32)
    t_g = np.ascontiguousarray(t_g, dtype=np.float32)
    V = np.ascontiguousarray(V, dtype=np.float32)
    T = np.ascontiguousarray(T, dtype=np.float32)
    A_v = np.ascontiguousarray(A_v, dtype=np.float32)
    A_t = np.ascontiguousarray(A_t, dtype=np.float32)

    # host-precomputed InfoNCE normalization scales (tiny)
    rv = (
        1.0 / np.maximum(np.sqrt((v_g.astype(np.float64) ** 2).sum(1)), 1e-12)
    ).astype(np.float32)[:, None]
    rt = (
        1.0 / np.maximum(np.sqrt((t_g.astype(np.float64) ** 2).sum(1)), 1e-12)
    ).astype(np.float32)[:, None]

    in_maps = []
    for c in range(NCORES):
        sl = slice(c * BLOC, (c + 1) * BLOC)
        in_maps.append(
            {
                "v_loc": V[sl],
                "t_loc": T[sl],
                "av_loc": A_v[sl],
                "at_loc": A_t[sl],
                "vg": v_g,
                "tg": t_g,
                "rv": rv,
                "rt": rt,
            }
        )

    res = run_bass_kernel_spmd(nc, in_maps, core_ids=list(range(NCORES)))
    results = res.results
    _CACHE["last_exec_time_ns"] = res.exec_time_ns
    _CACHE["last_profile_json"] = res.profile_json
    _CACHE["last_insts_trace"] = res.instructions_and_trace

    # ----- gather -----
    S = np.concatenate(
        [
            results[c]["s_out"].reshape(N, BLOC, N).transpose(1, 0, 2)
            for c in range(NCORES)
        ]
    ).astype(np.float64)  # [B, N, N], S[b,n,m] = V[b,n] . T[b,m]
    rsq_v = np.concatenate(
        [results[c]["rsq_out"][:, :BLOC].T for c in range(NCORES)]
    ).astype(np.float64)
    rsq_t = np.concatenate(
        [results[c]["rsq_out"][:, BLOC:].T for c in range(NCORES)]
    ).astype(np.float64)

    # ----- Hungarian on host (same as reference) -----
    inv_nv = 1.0 / np.maximum(np.sqrt(rsq_v), 1e-12)  # [B, N]
    inv_nt = 1.0 / np.maximum(np.sqrt(rsq_t), 1e-12)
    cost = 1.0 - S * inv_nv[:, :, None] * inv_nt[:, None, :]
    cols = _solve_hungarian(cost)  # [B, N]

    # cross = sum_b sum_n S[b, n, cols[b, n]]
    cross = S[np.arange(B)[:, None], np.arange(N)[None, :], cols].sum()
    L_node = (rsq_v.sum() + rsq_t.sum() - 2.0 * cross) / float(B * N * D)

    # ----- graph term -----
    L_graph = (
        sum(results[c]["apart_out"].astype(np.float64).sum() for c in range(NCORES))
        / float(B * N * N)
    )

    # ----- global InfoNCE (replicated; use core 0) -----
    nce = results[0]["nce_out"].astype(np.float64)  # [B,4]: lse_i, diag, lse_t
    lse_i, diag, lse_t = nce[:, 0], nce[:, 1], nce[:, 2]
    l_i2t = np.mean(lse_i - diag)
    l_t2i = np.mean(lse_t - diag)
    L_global = 0.5 * (l_i2t + l_t2i)

    total = L_global + L_node + L_graph
    return (
        np.float32(total),
        np.float32(L_global),
        np.float32(L_node),
        np.float32(L_graph),
    )
